# revision 45
# baseline (speedup 1.0000x reference)
"""Single-head causal self-attention on 8 Trainium2 NeuronCores.

Problem: x[8, 4096, 1024], Wq/Wk/Wv[1024, 128] ->
  out[b] = softmax(causal((x[b] @ Wq) @ (x[b] @ Wk)^T / sqrt(128))) @ (x[b] @ Wv)

Sharding: data-parallel over batch -- each of the 8 cores handles one batch
element. Inputs are fed per-core as xT = x[b].T (layout prep on host) so the
contraction dim C lands on SBUF partitions.

Design (fp16 end-to-end on chip, fp32 PSUM accumulation):
  - scores kept transposed [kv, q]; PV consumes exp tiles directly
    (lhsT = v-natural block, rhs = exp tile) -> no transposes in phase 2.
  - attention emitted with one block of scores lookahead
    (s(k+1) before PV(k)) so the scalar engine's exp latency is hidden.
  - QKV projection t-groups interleaved with attention q-groups; the last
    t-groups are fed matmul-by-matmul into the attention block stream
    (att1 gets tg4/tg5, att2 gets tg6/tg7) to fill exp-wait slots.
  - PE p-state warmup matmuls on memset tiles run during the initial DMAs.
  - softmax denominator: exp tiles accumulated on DVE in fp16 (2-byte 4x
    mode) into two parity accumulators, reduced across partitions by a
    ones-vector matmul accumulating both accs in PSUM.
  - numerator shipped per 256/512-col region as soon as its accumulation
    stops (fp16, scaled 1/16), denominators DMA'd raw; division on host.
  - PSUM budget exactly 8 banks: qkv tag 2x1, att tag (scores/transpose/
    denominator rotation) 2x2, output accumulator 1x2.
"""

from collections import deque
from functools import partial

import numpy as np

import concourse.bass as bass
import concourse.tile as tile
from concourse import bacc, mybir
from concourse.bass_utils import run_bass_kernel_spmd

B, T, C, HS = 8, 4096, 1024, 128
P = 128
NCORES = 8
CCH = C // P            # 8 c-chunks
TG = T // 512           # 8 t-groups (phase 1 granularity)
QG = T // 1024          # 4 q-groups (phase 2 granularity)
OSCALE = 1.0 / 16.0     # on-chip numerator scale so fp16 output can't overflow
SCALE = float(HS) ** -0.5

f32 = mybir.dt.float32
f16 = mybir.dt.float16
EXP = mybir.ActivationFunctionType.Exp

_NC = None


def build_program():
    nc = bacc.Bacc()
    xT = nc.declare_dram_parameter("xT", [C, T], f16, isOutput=False)
    # weights host-prepacked to [p, j, d] (j = c-chunk)
    Wq = nc.declare_dram_parameter("Wq", [P, CCH, HS], f16, isOutput=False)
    Wk = nc.declare_dram_parameter("Wk", [P, CCH, HS], f16, isOutput=False)
    Wv = nc.declare_dram_parameter("Wv", [P, CCH, HS], f16, isOutput=False)
    # host constants: [ones(2) | identity(128) | trimask(128)] all fp16
    aux = nc.declare_dram_parameter("aux", [P, 258], f16, isOutput=False)
    # unnormalized attention output (numerator, scaled by OSCALE) and
    # softmax denominators; the division happens on host
    outT = nc.declare_dram_parameter("outT", [HS, T], f16, isOutput=True)
    den = nc.declare_dram_parameter("den", [QG, 1024], f32, isOutput=True)

    xT_r = xT[:].rearrange("(j p) t -> p j t", p=P)

    with tile.TileContext(nc) as tc:
        with (
            tc.tile_pool(name="consts", bufs=1) as consts,
            tc.tile_pool(name="big", bufs=1) as big,
            tc.tile_pool(name="xin", bufs=3) as xin,
            tc.tile_pool(name="vtp", bufs=2) as vtp,
            tc.tile_pool(name="ptp", bufs=4) as ptp,
            tc.tile_pool(name="accp", bufs=2) as accp,
            tc.tile_pool(name="ocnp", bufs=2) as ocnp,
            tc.tile_pool(name="ps_qkv", bufs=2, space="PSUM") as ps_qkv,
            tc.tile_pool(name="ps_att", bufs=2, space="PSUM") as ps_att,
            tc.tile_pool(name="ps_o", bufs=1, space="PSUM") as ps_o,
        ):
            # PE p-state warmup: throwaway matmuls on memset tiles while the
            # first input DMAs are in flight.
            wd = consts.tile([P, P], f16, tag="wd", name="wd")
            xd = consts.tile([P, 512], f16, tag="xd", name="xd")
            nc.vector.memset(wd[:], 0.0)
            nc.vector.memset(xd[:], 0.0)
            dps = ps_qkv.tile([P, 512], f32, tag="ps", name="dps")
            for i in range(8):
                nc.tensor.matmul(dps[:], lhsT=wd[:], rhs=xd[:],
                                 start=(i == 0), stop=(i == 7))

            w_sb = [consts.tile([P, CCH, HS], f16, tag=f"w{i}", name=f"w{i}")
                    for i in range(3)]
            # Wq chunk 0 first so the very first projection matmul can start,
            # then the first x chunks, then the bulk of the weights.
            nc.sync.dma_start(out=w_sb[0][:, 0:1, :], in_=Wq[:, 0:1, :])
            aux_sb = consts.tile([P, 258], f16)
            ones = aux_sb[:, 0:2]
            ident = aux_sb[:, 2:130]
            trimask = aux_sb[:, 130:258]

            qT = big.tile([P, T], f16, tag="qT")   # [d, t]
            kT = big.tile([P, T], f16, tag="kT")   # [d, t]
            vS = big.tile([P, T], f16, tag="vS")   # [t%128, 128*blk + d]

            def qkv_units(tg):
                """Issue the x DMAs for t-group tg now; return a list of
                emission thunks (24 matmuls + 3 copies) and the vt tile."""
                t0 = 512 * tg
                xts = [xin.tile([P, 512], f16, tag=f"xt{j}", name=f"xt{j}")
                       for j in range(CCH)]
                for j in range(CCH):
                    nc.sync.dma_start(out=xts[j][:], in_=xT_r[:, j, t0:t0 + 512])
                    if tg == 0 and j == 1:
                        # rest of the weights ride behind the first x chunks
                        nc.sync.dma_start(
                            out=w_sb[0][:, 1:CCH, :], in_=Wq[:, 1:CCH, :])
                        nc.sync.dma_start(out=w_sb[1][:], in_=Wk[:])
                        nc.sync.dma_start(out=w_sb[2][:], in_=Wv[:])
                vt = vtp.tile([P, 512], f16, tag="vt", name="vt")
                state = {}

                def mm(i, j):
                    if j == 0:
                        state[i] = ps_qkv.tile([P, 512], f32, tag="ps",
                                               name="ps")
                    nc.tensor.matmul(
                        state[i][:], lhsT=w_sb[i][:, j, :], rhs=xts[j][:],
                        start=(j == 0), stop=(j == CCH - 1))

                def cp(i):
                    ps = state.pop(i)
                    if i == 0:
                        nc.scalar.copy(qT[:, t0:t0 + 512], ps[:])
                    elif i == 1:
                        nc.vector.tensor_copy(kT[:, t0:t0 + 512], ps[:])
                    else:
                        nc.vector.tensor_copy(vt[:], ps[:])

                # Q and K chunk-major (two PSUM accumulators live at once):
                # each arriving x chunk feeds two matmuls, which keeps the PE
                # busy while the startup DMAs are still streaming in.
                units = []
                for j in range(CCH):
                    units.append(partial(mm, 0, j))
                    units.append(partial(mm, 1, j))
                units.append(partial(cp, 0))
                units.append(partial(cp, 1))
                for j in range(CCH):
                    units.append(partial(mm, 2, j))
                units.append(partial(cp, 2))
                return units, vt

            def run_all(units):
                for u in units:
                    u()

            def v_transpose(tg, vt):
                """PE-transpose vt [d, 512] into v-natural blocks of vS."""
                tr = ps_att.tile([P, 512], f16, tag="att", name="tr")
                for m in range(4):
                    nc.tensor.transpose(
                        tr[:, 128 * m:128 * (m + 1)],
                        vt[:, 128 * m:128 * (m + 1)], ident)
                nc.vector.tensor_copy(vS[:, 512 * tg:512 * tg + 512], tr[:])

            def att_group(g, feed=()):
                """Causal attention for q columns [1024*g, 1024*g+1024),
                interleaving `feed` thunks into the block stream."""
                feed = deque(feed)
                n_feed = len(feed)
                fed = 0
                q0 = 1024 * g
                o_ps = ps_o.tile([P, 1024], f32, tag="o_ps", name="o_ps")
                accs = [accp.tile([P, 1024], f16, tag=f"acc{a}", name=f"acc{a}")
                        for a in range(2)]
                nkv = 8 * (g + 1)

                def emit_scores(k):
                    vstart = max(0, 128 * k - q0)
                    s_ps = ps_att.tile([P, 1024], f32, tag="att", name="s_ps")
                    for c in range(2):
                        cq = 512 * c
                        lc = max(0, vstart - cq)
                        if lc >= 512:
                            continue
                        nc.tensor.matmul(
                            s_ps[:, cq + lc:cq + 512],
                            lhsT=kT[:, 128 * k:128 * (k + 1)],
                            rhs=qT[:, q0 + cq + lc:q0 + cq + 512],
                            start=True, stop=True,
                        )
                    return s_ps

                for k in range(nkv):
                    vstart = max(0, 128 * k - q0)
                    s_ps = emit_scores(k)
                    pt = ptp.tile([P, 1024], f16, tag="pt", name="pt")
                    nc.scalar.activation(
                        pt[:, vstart:1024], s_ps[:, vstart:1024], EXP,
                        scale=SCALE)
                    if k >= 8 * g:  # diagonal block: mask kv > q
                        nc.vector.tensor_mul(
                            pt[:, vstart:vstart + 128],
                            pt[:, vstart:vstart + 128], trimask)
                    acc = accs[k % 2]
                    if k < 2:
                        if vstart:
                            nc.vector.memset(acc[:, 0:vstart], 0.0)
                        nc.vector.tensor_copy(
                            acc[:, vstart:1024], pt[:, vstart:1024])
                    else:
                        nc.vector.tensor_add(
                            acc[:, vstart:1024], acc[:, vstart:1024],
                            pt[:, vstart:1024])
                    while fed < n_feed * (k + 1) // nkv:
                        feed.popleft()()
                        fed += 1
                    for c in range(2):
                        cq = 512 * c
                        lc = max(0, vstart - cq)
                        if lc >= 512:
                            continue
                        last_k = 8 * g + 4 * c + 3
                        nc.tensor.matmul(
                            o_ps[:, cq + lc:cq + 512],
                            lhsT=vS[:, 128 * k:128 * (k + 1)],
                            rhs=pt[:, cq + lc:cq + 512],
                            start=(k == 0), stop=(k == last_k),
                        )
                    # ship each output region as soon as its accumulation
                    # finished: [0:512] after k=8g+3, [512:768] after 8g+5,
                    # [768:1024] after 8g+7
                    for (kf, lo, hi) in ((3, 0, 512), (5, 512, 768),
                                         (7, 768, 1024)):
                        if k == 8 * g + kf:
                            ocn = ocnp.tile(
                                [P, hi - lo], f16, tag=f"ocn{kf}",
                                name=f"ocn{kf}")
                            nc.vector.tensor_scalar_mul(
                                ocn[:], o_ps[:, lo:hi], OSCALE)
                            nc.sync.dma_start(
                                out=outT[:, q0 + lo:q0 + hi], in_=ocn[:])

                # denominator = ones^T @ (acc0 + acc1) via PSUM accumulation
                dr_ps = ps_att.tile([2, 1024], f32, tag="att", name="dr_ps")
                for c in range(2):
                    for a in range(2):
                        nc.tensor.matmul(
                            dr_ps[:, 512 * c:512 * (c + 1)],
                            lhsT=ones, rhs=accs[a][:, 512 * c:512 * (c + 1)],
                            start=(a == 0), stop=(a == 1),
                        )
                drs = ocnp.tile([1, 1024], f32, tag="drs", name="drs")
                nc.vector.tensor_copy(drs[:], dr_ps[0:1, :])
                nc.sync.dma_start(out=den[g:g + 1, :], in_=drs[:])

            # ---- interleaved schedule ----
            u0, vt0 = qkv_units(0)
            run_all(u0)
            # aux is needed first by v_transpose(0); keep it off the startup
            # critical path (w + first x chunks)
            nc.sync.dma_start(out=aux_sb[:], in_=aux[:])
            u1, vt1 = qkv_units(1)
            run_all(u1)
            v_transpose(0, vt0)
            v_transpose(1, vt1)
            att_group(0)
            u2, vt2 = qkv_units(2)
            run_all(u2)
            u3, vt3 = qkv_units(3)
            run_all(u3)
            v_transpose(2, vt2)
            v_transpose(3, vt3)
            att_group(1)
            u4, vt4 = qkv_units(4)
            run_all(u4)
            v_transpose(4, vt4)
            u5, vt5 = qkv_units(5)
            run_all(u5)
            v_transpose(5, vt5)
            att_group(2)
            u6, vt6 = qkv_units(6)
            run_all(u6)
            v_transpose(6, vt6)
            u7, vt7 = qkv_units(7)
            run_all(u7)
            v_transpose(7, vt7)
            att_group(3)

    nc.finalize()
    return nc


def _get_nc():
    global _NC
    if _NC is None:
        _NC = build_program()
    return _NC


def make_aux():
    aux = np.zeros((P, 258), dtype=np.float16)
    aux[:, 0:2] = 1.0
    aux[:, 2:130] = np.eye(P, dtype=np.float16)
    aux[:, 130:258] = np.triu(np.ones((P, P), dtype=np.float16))
    return aux


def _prep_w(W):
    """[C, HS] -> [P, CCH, HS] with W_p[p, j, d] = W[j*128+p, d]."""
    w = np.asarray(W, dtype=np.float32).astype(np.float16)
    return np.ascontiguousarray(w.reshape(CCH, P, HS).transpose(1, 0, 2))


def make_in_maps(x, Wq, Wk, Wv):
    x = np.asarray(x, dtype=np.float32)
    aux = make_aux()
    wq, wk, wv = _prep_w(Wq), _prep_w(Wk), _prep_w(Wv)
    return [
        {
            "xT": np.ascontiguousarray(x[b].T.astype(np.float16)),
            "Wq": wq,
            "Wk": wk,
            "Wv": wv,
            "aux": aux,
        }
        for b in range(NCORES)
    ]


def finish_host(res):
    """Divide numerators by softmax denominators and assemble [B, T, HS]."""
    out = np.empty((NCORES, T, HS), dtype=np.float32)
    inv_oscale = 1.0 / OSCALE
    for b in range(NCORES):
        numer = res.results[b]["outT"].T.astype(np.float32)  # [T, HS]
        d = res.results[b]["den"].reshape(T, 1)              # [T, 1] fp32
        out[b] = numer * (inv_oscale / d)
    return out


def kernel(x, Wq, Wk, Wv):
    assert x.shape == (B, T, C) and Wq.shape == (C, HS)
    nc = _get_nc()
    in_maps = make_in_maps(x, Wq, Wk, Wv)
    res = run_bass_kernel_spmd(nc, in_maps, list(range(NCORES)))
    return finish_host(res)


# revision 53
# speedup vs baseline: 1.0128x; 1.0128x over previous
"""Single-head causal self-attention on 8 Trainium2 NeuronCores.

Problem: x[8, 4096, 1024], Wq/Wk/Wv[1024, 128] ->
  out[b] = softmax(causal((x[b] @ Wq) @ (x[b] @ Wk)^T / sqrt(128))) @ (x[b] @ Wv)

Sharding: data-parallel over batch -- each of the 8 cores handles one batch
element. Inputs are fed per-core as xT = x[b].T (layout prep on host) so the
contraction dim C lands on SBUF partitions.

Design (fp16 end-to-end on chip, fp32 PSUM accumulation):
  - scores kept transposed [kv, q]; PV consumes exp tiles directly
    (lhsT = v-natural block, rhs = exp tile) -> no transposes in phase 2.
  - attention emitted with one block of scores lookahead
    (s(k+1) before PV(k)) so the scalar engine's exp latency is hidden.
  - QKV projection t-groups interleaved with attention q-groups; the last
    t-groups are fed matmul-by-matmul into the attention block stream
    (att1 gets tg4/tg5, att2 gets tg6/tg7) to fill exp-wait slots.
  - PE p-state warmup matmuls on memset tiles run during the initial DMAs.
  - softmax denominator: exp tiles accumulated on DVE in fp16 (2-byte 4x
    mode) into two parity accumulators, reduced across partitions by a
    ones-vector matmul accumulating both accs in PSUM.
  - numerator shipped per 256/512-col region as soon as its accumulation
    stops (fp16, scaled 1/16), denominators DMA'd raw; division on host.
  - PSUM budget exactly 8 banks: qkv tag 2x1, att tag (scores/transpose/
    denominator rotation) 2x2, output accumulator 1x2.
"""

from collections import deque
from functools import partial

import numpy as np

import concourse.bass as bass
import concourse.tile as tile
from concourse import bacc, mybir
from concourse.bass_utils import run_bass_kernel_spmd

B, T, C, HS = 8, 4096, 1024, 128
P = 128
NCORES = 8
CCH = C // P            # 8 c-chunks
TG = T // 512           # 8 t-groups (phase 1 granularity)
QG = T // 1024          # 4 q-groups (phase 2 granularity)
OSCALE = 1.0 / 16.0     # on-chip numerator scale so fp16 output can't overflow
SCALE = float(HS) ** -0.5

f32 = mybir.dt.float32
f16 = mybir.dt.float16
EXP = mybir.ActivationFunctionType.Exp

_NC = None


def build_program():
    nc = bacc.Bacc()
    xT = nc.declare_dram_parameter("xT", [C, T], f16, isOutput=False)
    # weights host-prepacked to [p, j, d] (j = c-chunk)
    Wq = nc.declare_dram_parameter("Wq", [P, CCH, HS], f16, isOutput=False)
    Wk = nc.declare_dram_parameter("Wk", [P, CCH, HS], f16, isOutput=False)
    Wv = nc.declare_dram_parameter("Wv", [P, CCH, HS], f16, isOutput=False)
    # host constants: [ones(2) | identity(128) | trimask(128)] all fp16
    aux = nc.declare_dram_parameter("aux", [P, 258], f16, isOutput=False)
    # unnormalized attention output (numerator, scaled by OSCALE) and
    # softmax denominators; the division happens on host
    outT = nc.declare_dram_parameter("outT", [HS, T], f16, isOutput=True)
    den = nc.declare_dram_parameter("den", [QG, 1024], f32, isOutput=True)

    xT_r = xT[:].rearrange("(j p) t -> p j t", p=P)

    with tile.TileContext(nc) as tc:
        with (
            tc.tile_pool(name="consts", bufs=1) as consts,
            tc.tile_pool(name="big", bufs=1) as big,
            tc.tile_pool(name="xin", bufs=3) as xin,
            tc.tile_pool(name="vtp", bufs=2) as vtp,
            tc.tile_pool(name="ptp", bufs=4) as ptp,
            tc.tile_pool(name="accp", bufs=2) as accp,
            tc.tile_pool(name="ocnp", bufs=2) as ocnp,
            tc.tile_pool(name="ps_qkv", bufs=2, space="PSUM") as ps_qkv,
            tc.tile_pool(name="ps_att", bufs=2, space="PSUM") as ps_att,
            tc.tile_pool(name="ps_o", bufs=1, space="PSUM") as ps_o,
        ):
            # PE p-state warmup: throwaway matmuls on memset tiles while the
            # first input DMAs are in flight.
            wd = consts.tile([P, P], f16, tag="wd", name="wd")
            xd = consts.tile([P, 512], f16, tag="xd", name="xd")
            nc.vector.memset(wd[:], 0.0)
            nc.vector.memset(xd[:], 0.0)
            dps = ps_qkv.tile([P, 512], f32, tag="ps", name="dps")
            for i in range(8):
                nc.tensor.matmul(dps[:], lhsT=wd[:], rhs=xd[:],
                                 start=(i == 0), stop=(i == 7))

            w_sb = [consts.tile([P, CCH, HS], f16, tag=f"w{i}", name=f"w{i}")
                    for i in range(3)]
            # Wq chunk 0 first so the very first projection matmul can start,
            # then the first x chunks, then the bulk of the weights.
            nc.sync.dma_start(out=w_sb[0][:, 0:1, :], in_=Wq[:, 0:1, :])
            aux_sb = consts.tile([P, 258], f16)
            ones = aux_sb[:, 0:2]
            ident = aux_sb[:, 2:130]
            trimask = aux_sb[:, 130:258]

            qT = big.tile([P, T], f16, tag="qT")   # [d, t]
            kT = big.tile([P, T], f16, tag="kT")   # [d, t]
            vS = big.tile([P, T], f16, tag="vS")   # [t%128, 128*blk + d]

            def qkv_units(tg):
                """Issue the x DMAs for t-group tg now; return a list of
                emission thunks (24 matmuls + 3 copies) and the vt tile."""
                t0 = 512 * tg
                xts = [xin.tile([P, 512], f16, tag=f"xt{j}", name=f"xt{j}")
                       for j in range(CCH)]
                for j in range(CCH):
                    nc.sync.dma_start(out=xts[j][:], in_=xT_r[:, j, t0:t0 + 512])
                    if tg == 0 and j == 1:
                        # rest of the weights ride behind the first x chunks
                        nc.sync.dma_start(
                            out=w_sb[0][:, 1:CCH, :], in_=Wq[:, 1:CCH, :])
                        nc.sync.dma_start(out=w_sb[1][:], in_=Wk[:])
                        nc.sync.dma_start(out=w_sb[2][:], in_=Wv[:])
                vt = vtp.tile([P, 512], f16, tag="vt", name="vt")
                state = {}

                def mm(i, j):
                    if j == 0:
                        state[i] = ps_qkv.tile([P, 512], f32, tag="ps",
                                               name="ps")
                    nc.tensor.matmul(
                        state[i][:], lhsT=w_sb[i][:, j, :], rhs=xts[j][:],
                        start=(j == 0), stop=(j == CCH - 1))

                def cp(i):
                    ps = state.pop(i)
                    if i == 0:
                        nc.scalar.copy(qT[:, t0:t0 + 512], ps[:])
                    elif i == 1:
                        nc.vector.tensor_copy(kT[:, t0:t0 + 512], ps[:])
                    else:
                        nc.vector.tensor_copy(vt[:], ps[:])

                units = []
                for i in range(3):
                    for j in range(CCH):
                        units.append(partial(mm, i, j))
                    units.append(partial(cp, i))
                return units, vt

            def run_all(units):
                for u in units:
                    u()

            def v_transpose(tg, vt):
                """PE-transpose vt [d, 512] into v-natural blocks of vS."""
                tr = ps_att.tile([P, 512], f16, tag="att", name="tr")
                for m in range(4):
                    nc.tensor.transpose(
                        tr[:, 128 * m:128 * (m + 1)],
                        vt[:, 128 * m:128 * (m + 1)], ident)
                nc.vector.tensor_copy(vS[:, 512 * tg:512 * tg + 512], tr[:])

            def att_group(g, feed=()):
                """Causal attention for q columns [1024*g, 1024*g+1024),
                interleaving `feed` thunks into the block stream."""
                feed = deque(feed)
                n_feed = len(feed)
                fed = 0
                q0 = 1024 * g
                o_ps = ps_o.tile([P, 1024], f32, tag="o_ps", name="o_ps")
                accs = [accp.tile([P, 1024], f16, tag=f"acc{a}", name=f"acc{a}")
                        for a in range(2)]
                nkv = 8 * (g + 1)

                def emit_scores(k):
                    vstart = max(0, 128 * k - q0)
                    s_ps = ps_att.tile([P, 1024], f32, tag="att", name="s_ps")
                    for c in range(2):
                        cq = 512 * c
                        lc = max(0, vstart - cq)
                        if lc >= 512:
                            continue
                        nc.tensor.matmul(
                            s_ps[:, cq + lc:cq + 512],
                            lhsT=kT[:, 128 * k:128 * (k + 1)],
                            rhs=qT[:, q0 + cq + lc:q0 + cq + 512],
                            start=True, stop=True,
                        )
                    return s_ps

                sps = emit_scores(0)
                for k in range(nkv):
                    vstart = max(0, 128 * k - q0)
                    s_ps = sps
                    pt = ptp.tile([P, 1024], f16, tag="pt", name="pt")
                    nc.scalar.activation(
                        pt[:, vstart:1024], s_ps[:, vstart:1024], EXP,
                        scale=SCALE)
                    if k >= 8 * g:  # diagonal block: mask kv > q
                        nc.vector.tensor_mul(
                            pt[:, vstart:vstart + 128],
                            pt[:, vstart:vstart + 128], trimask)
                    acc = accs[k % 2]
                    if k < 2:
                        if vstart:
                            nc.vector.memset(acc[:, 0:vstart], 0.0)
                        nc.vector.tensor_copy(
                            acc[:, vstart:1024], pt[:, vstart:1024])
                    else:
                        nc.vector.tensor_add(
                            acc[:, vstart:1024], acc[:, vstart:1024],
                            pt[:, vstart:1024])
                    # next block's scores go to the PE before PV(k), giving
                    # the exp above time to finish off the critical path
                    if k + 1 < nkv:
                        sps = emit_scores(k + 1)
                    while fed < n_feed * (k + 1) // nkv:
                        feed.popleft()()
                        fed += 1
                    for c in range(2):
                        cq = 512 * c
                        lc = max(0, vstart - cq)
                        if lc >= 512:
                            continue
                        last_k = 8 * g + 4 * c + 3
                        nc.tensor.matmul(
                            o_ps[:, cq + lc:cq + 512],
                            lhsT=vS[:, 128 * k:128 * (k + 1)],
                            rhs=pt[:, cq + lc:cq + 512],
                            start=(k == 0), stop=(k == last_k),
                        )
                    # ship each output region as soon as its accumulation
                    # finished: [0:512] after k=8g+3, [512:768] after 8g+5,
                    # [768:1024] after 8g+7
                    for (kf, lo, hi) in ((3, 0, 512), (5, 512, 768),
                                         (7, 768, 1024)):
                        if k == 8 * g + kf:
                            ocn = ocnp.tile(
                                [P, hi - lo], f16, tag=f"ocn{kf}",
                                name=f"ocn{kf}")
                            nc.vector.tensor_scalar_mul(
                                ocn[:], o_ps[:, lo:hi], OSCALE)
                            # last group's final region: issue from the Act
                            # hwdge queue (idle at the tail, Sync may have
                            # backlog)
                            eng = (nc.scalar if g == QG - 1 and kf == 7
                                   else nc.sync)
                            eng.dma_start(
                                out=outT[:, q0 + lo:q0 + hi], in_=ocn[:])

                # denominator = ones^T @ (acc0 + acc1) via PSUM accumulation
                dr_ps = ps_att.tile([2, 1024], f32, tag="att", name="dr_ps")
                for c in range(2):
                    for a in range(2):
                        nc.tensor.matmul(
                            dr_ps[:, 512 * c:512 * (c + 1)],
                            lhsT=ones, rhs=accs[a][:, 512 * c:512 * (c + 1)],
                            start=(a == 0), stop=(a == 1),
                        )
                # drs copy on Act (idle after the group's last exp) so it
                # runs in parallel with the DVE region mul in the tail
                drs = ocnp.tile([1, 1024], f32, tag="drs", name="drs")
                nc.scalar.copy(drs[:], dr_ps[0:1, :])
                eng = nc.scalar if g == QG - 1 else nc.sync
                eng.dma_start(out=den[g:g + 1, :], in_=drs[:])

            # ---- interleaved schedule ----
            u0, vt0 = qkv_units(0)
            run_all(u0)
            # aux is needed first by v_transpose(0); keep it off the startup
            # critical path (w + first x chunks)
            nc.sync.dma_start(out=aux_sb[:], in_=aux[:])
            u1, vt1 = qkv_units(1)
            run_all(u1)
            v_transpose(0, vt0)
            v_transpose(1, vt1)
            att_group(0)
            u2, vt2 = qkv_units(2)
            run_all(u2)
            u3, vt3 = qkv_units(3)
            run_all(u3)
            v_transpose(2, vt2)
            v_transpose(3, vt3)
            att_group(1)
            u4, vt4 = qkv_units(4)
            run_all(u4)
            v_transpose(4, vt4)
            u5, vt5 = qkv_units(5)
            run_all(u5)
            v_transpose(5, vt5)
            att_group(2)
            u6, vt6 = qkv_units(6)
            run_all(u6)
            v_transpose(6, vt6)
            u7, vt7 = qkv_units(7)
            run_all(u7)
            v_transpose(7, vt7)
            att_group(3)

    nc.finalize()
    return nc


def _get_nc():
    global _NC
    if _NC is None:
        _NC = build_program()
    return _NC


def make_aux():
    aux = np.zeros((P, 258), dtype=np.float16)
    aux[:, 0:2] = 1.0
    aux[:, 2:130] = np.eye(P, dtype=np.float16)
    aux[:, 130:258] = np.triu(np.ones((P, P), dtype=np.float16))
    return aux


def _prep_w(W):
    """[C, HS] -> [P, CCH, HS] with W_p[p, j, d] = W[j*128+p, d]."""
    w = np.asarray(W, dtype=np.float32).astype(np.float16)
    return np.ascontiguousarray(w.reshape(CCH, P, HS).transpose(1, 0, 2))


def make_in_maps(x, Wq, Wk, Wv):
    x = np.asarray(x, dtype=np.float32)
    aux = make_aux()
    wq, wk, wv = _prep_w(Wq), _prep_w(Wk), _prep_w(Wv)
    return [
        {
            "xT": np.ascontiguousarray(x[b].T.astype(np.float16)),
            "Wq": wq,
            "Wk": wk,
            "Wv": wv,
            "aux": aux,
        }
        for b in range(NCORES)
    ]


def finish_host(res):
    """Divide numerators by softmax denominators and assemble [B, T, HS]."""
    out = np.empty((NCORES, T, HS), dtype=np.float32)
    inv_oscale = 1.0 / OSCALE
    for b in range(NCORES):
        numer = res.results[b]["outT"].T.astype(np.float32)  # [T, HS]
        d = res.results[b]["den"].reshape(T, 1)              # [T, 1] fp32
        out[b] = numer * (inv_oscale / d)
    return out


def kernel(x, Wq, Wk, Wv):
    assert x.shape == (B, T, C) and Wq.shape == (C, HS)
    nc = _get_nc()
    in_maps = make_in_maps(x, Wq, Wk, Wv)
    res = run_bass_kernel_spmd(nc, in_maps, list(range(NCORES)))
    return finish_host(res)
